# revision 1
# baseline (speedup 1.0000x reference)
"""Multi-head attention (B=16, N=1024, D=512, H=8) on 8 TRN2 NeuronCores.

Strategy: pure data-parallel over batch (2 batches/core, no collectives).
All input transposes (x.T, W.T, mask.T) are done host-side in numpy.
On-device, per core:
  phase 1: Q^T,K^T [channel, token] and V [token, channel] projections (bf16)
  phase 2: k-major attention per (head-pair, batch):
           E^T tile [k=128, q=1024] = K^T_h.T @ Q^T_h  (K=64 contraction)
           exp via ACT (scale=1/8 folded in), no max-subtraction (logits ~N(0,1))
           denominator via ones[128,64]-lhsT matmul -> broadcast rows in PSUM
           mask multiply on DVE (bf16 2x), PV matmul accumulates head_out^T
           epilogue: reciprocal + fused normalize into A^T (concat-head) buffer
  phase 3: output projection from A^T, bias add, DMA out [n,o] row-major.
"""

import os

import numpy as np
import ml_dtypes

import concourse.bass as bass
import concourse.mybir as mybir
import concourse.tile as tile
from concourse.bass_utils import run_bass_kernel_spmd
from concourse.vector_clock import ScopedClock

# ---------------------------------------------------------------------------
# Workaround: the walrus build in this container only supports ONE sync-wait
# command per instruction, but this Tile snapshot emits instructions carrying
# several (e.g. an activation waiting on both a PE sem and a DMA-lane sem,
# and the kernel-tail drain waiting on all procs).  Split surplus waits onto
# preceding same-engine InstEventSemaphore carriers, which is semantically
# identical (the engine blocks on each wait in stream order).
# ---------------------------------------------------------------------------
_orig_commit = tile.TileContext._commit_instruction


def _split_commit(self, inst, lazy_reg_writes=True):
    si = inst.sync_info
    if (si is not None and si.on_wait and len(si.on_wait) > 1
            and inst.engine != mybir.EngineType.Unassigned):
        waits = list(si.on_wait)
        for w in waits[:-1]:
            es = mybir.InstEventSemaphore(
                name=self.nc.get_next_instruction_name(),
                engine=inst.engine, ins=[], outs=[],
                sync_info=mybir.SyncInfo(on_wait=[w], on_update=[]),
            )
            _orig_commit(self, es, lazy_reg_writes=False)
        inst.sync_info = mybir.SyncInfo(
            on_wait=[waits[-1]], on_update=list(si.on_update or []))
    return _orig_commit(self, inst, lazy_reg_writes)


def _patched_drain_and_barrier(self, tick_clock, wait_clock):
    drain_inst = self.nc.sync.drain()
    wait_clock.add_sem_waits(
        drain_inst.ins, ScopedClock({None: tick_clock.global_clock}))
    si = drain_inst.ins.sync_info
    if si is not None and si.on_wait and len(si.on_wait) > 1:
        waits = list(si.on_wait)
        drain_inst.ins.sync_info = mybir.SyncInfo(
            on_wait=[waits[0]], on_update=list(si.on_update or []))
        for w in waits[1:]:
            es = mybir.InstEventSemaphore(
                name=self.nc.get_next_instruction_name(),
                engine=mybir.EngineType.SP, ins=[], outs=[],
                sync_info=mybir.SyncInfo(on_wait=[w], on_update=[]),
            )
            self._add_instruction(es)
    self.nc.all_engine_barrier()
    assert self.sems is not None
    popped = self.nc._tile_sem_poison_stack.pop()
    assert popped is self._sem_poison
    self.nc.clear_and_free_semaphores(list(self.sems.allocated().values()))
    self.nc.all_engine_barrier()


tile.TileContext._commit_instruction = _split_commit
tile.TileContext._drain_and_barrier = _patched_drain_and_barrier

P = 128
NB = 2            # batches per core
N = 1024          # sequence length
D = 512           # model dim
H = 8             # heads
HD = 64           # head dim
T = NB * N        # tokens per core
DC = D // P       # channel chunks (4)
KT = N // P       # k tiles per batch (8)
TC = T // P       # token chunks (16)
NCORES = 8

BF = mybir.dt.bfloat16
F32 = mybir.dt.float32
AF = mybir.ActivationFunctionType
ALU = mybir.AluOpType

_cache = {}


def _build(reps=1):
    nc = bass.Bass()

    xT_d = nc.declare_dram_parameter("xT", [D, T], BF, isOutput=False)
    wq_d = nc.declare_dram_parameter("WqT", [D, D], BF, isOutput=False)
    wk_d = nc.declare_dram_parameter("WkT", [D, D], BF, isOutput=False)
    wv_d = nc.declare_dram_parameter("WvT", [D, D], BF, isOutput=False)
    wp_d = nc.declare_dram_parameter("WpT", [D, D], BF, isOutput=False)
    mk_d = nc.declare_dram_parameter("maskT", [N, N], BF, isOutput=False)
    bq_d = nc.declare_dram_parameter("bq2", [P, DC], F32, isOutput=False)
    bk_d = nc.declare_dram_parameter("bk2", [P, DC], F32, isOutput=False)
    bv_d = nc.declare_dram_parameter("bv_rep", [P, D], F32, isOutput=False)
    bp_d = nc.declare_dram_parameter("bp_rep", [P, D], F32, isOutput=False)
    out_d = nc.declare_dram_parameter("out", [NB, N, D], F32, isOutput=True)

    with tile.TileContext(nc) as tc:
        with tc.tile_pool(name="const", bufs=1) as const:
            # resident inputs
            xT_sb = const.tile([P, DC, T], BF)
            nc.sync.dma_start(xT_sb, xT_d[:].rearrange("(c p) t -> p c t", p=P))
            wq_sb = const.tile([P, DC, D], BF)
            nc.sync.dma_start(wq_sb, wq_d[:].rearrange("(c p) o -> p c o", p=P))
            wk_sb = const.tile([P, DC, D], BF)
            nc.sync.dma_start(wk_sb, wk_d[:].rearrange("(c p) o -> p c o", p=P))
            wv_sb = const.tile([P, DC, D], BF)
            nc.sync.dma_start(wv_sb, wv_d[:].rearrange("(c p) o -> p c o", p=P))
            wp_sb = const.tile([P, DC, D], BF)
            nc.sync.dma_start(wp_sb, wp_d[:].rearrange("(c p) o -> p c o", p=P))
            maskT_sb = const.tile([P, KT, N], BF)
            nc.sync.dma_start(maskT_sb, mk_d[:].rearrange("(k p) q -> p k q", p=P))
            bq_sb = const.tile([P, DC], F32)
            nc.sync.dma_start(bq_sb, bq_d[:])
            bk_sb = const.tile([P, DC], F32)
            nc.sync.dma_start(bk_sb, bk_d[:])
            bv_sb = const.tile([P, D], F32)
            nc.sync.dma_start(bv_sb, bv_d[:])
            bp_sb = const.tile([P, D], F32)
            nc.sync.dma_start(bp_sb, bp_d[:])

            ones64 = const.tile([P, HD], BF)
            nc.any.memset(ones64, 1.0)

            # resident intermediates
            QT_sb = const.tile([P, DC, T], BF)   # [chan, oc, token]
            KT_sb = const.tile([P, DC, T], BF)
            V_sb = const.tile([P, TC, D], BF)    # [token, tc, chan]
            A_sb = const.tile([P, NB * DC, N], BF)  # concat-head out^T per b

            def emit_phases():
                # Projections are interleaved with attention per head-pair:
                # V first, then for each oc: Q/K projections of that chunk
                # followed by its attention, so ACT/DVE pipelines fill while
                # later projections still stream on PE.
                with (
                    tc.tile_pool(name="pr_ps", bufs=2, space="PSUM") as pr_ps,
                    tc.tile_pool(name="et_ps", bufs=2, space="PSUM") as et_ps,
                    tc.tile_pool(name="pv_ps", bufs=1, space="PSUM") as pv_ps,
                    tc.tile_pool(name="dn_ps", bufs=1, space="PSUM") as dn_ps,
                    tc.tile_pool(name="sb", bufs=10) as sb,
                    tc.tile_pool(name="sb2", bufs=4) as sb2,
                ):
                    def emit_v_proj():
                        for t16 in range(TC):
                            ps = pr_ps.tile([P, D], F32, name="prproj")
                            for ic in range(DC):
                                nc.tensor.matmul(
                                    ps,
                                    lhsT=xT_sb[:, ic, t16 * P:(t16 + 1) * P],
                                    rhs=wv_sb[:, ic, :],
                                    start=(ic == 0),
                                    stop=(ic == DC - 1),
                                )
                            nc.vector.tensor_tensor(V_sb[:, t16, :], ps,
                                                    bv_sb, ALU.add)

                    def emit_qk_proj(oc):
                        for w_sb, b_sb, dst in ((wq_sb, bq_sb, QT_sb),
                                                (wk_sb, bk_sb, KT_sb)):
                            for ns in range(T // 512):
                                ps = pr_ps.tile([P, D], F32, name="prproj")
                                for ic in range(DC):
                                    nc.tensor.matmul(
                                        ps,
                                        lhsT=w_sb[:, ic, oc * P:(oc + 1) * P],
                                        rhs=xT_sb[:, ic, ns * 512:(ns + 1) * 512],
                                        start=(ic == 0),
                                        stop=(ic == DC - 1),
                                    )
                                nc.scalar.activation(
                                    dst[:, oc, ns * 512:(ns + 1) * 512],
                                    ps,
                                    AF.Identity,
                                    bias=b_sb[:, oc:oc + 1],
                                )
                    def emit_qk(b, hp, qh, kt):
                        # both heads' E^T slices in one 2-bank tile; the two
                        # matmuls use disjoint PE row-groups and banks.
                        et = et_ps.tile([P, 2, 512], F32, name="et")
                        for sub in range(2):
                            po = sub * HD
                            nc.tensor.matmul(
                                et[:, sub, :],
                                lhsT=KT_sb[po:po + HD, hp,
                                           b * N + kt * P: b * N + (kt + 1) * P],
                                rhs=QT_sb[po:po + HD, hp,
                                          b * N + qh * 512: b * N + (qh + 1) * 512],
                                start=True,
                                stop=True,
                            )
                        return et

                    def emit_exp_mask(et, qh, kt):
                        # one FD=1024 exp over both heads; one mask multiply
                        # with the mask row broadcast across the head axis.
                        ex = sb.tile([P, 2, 512], BF, name="ex")
                        nc.scalar.activation(ex, et, AF.Exp, scale=0.125)
                        mk = sb.tile([P, 2, 512], BF, name="mk")
                        nc.vector.tensor_tensor(
                            mk, ex,
                            maskT_sb[:, kt, None,
                                     qh * 512:(qh + 1) * 512].to_broadcast(
                                         (P, 2, 512)),
                            ALU.mult)
                        return ex, mk

                    def emit_dn_pv(b, hp, qh, kt, dnb, pv, ex, mk):
                        # dn: head 0 on PE col-groups 0-1, head 1 on 2-3;
                        # sub=1 first so its cross-engine wait covers sub=0
                        # and the pair issues back-to-back (co-runs).
                        for sub in (1, 0):
                            po = sub * HD
                            nc.tensor.matmul(
                                dnb[po:po + HD, :],
                                lhsT=ones64,
                                rhs=ex[:, sub, :],
                                start=(kt == 0),
                                stop=(kt == KT - 1),
                                tile_position=(0, po),
                            )
                        for sub in (1, 0):
                            po = sub * HD
                            nc.tensor.matmul(
                                pv[po:po + HD, :],
                                lhsT=V_sb[:, b * KT + kt,
                                          (2 * hp + sub) * HD:(2 * hp + sub + 1) * HD],
                                rhs=mk[:, sub, :],
                                start=(kt == 0),
                                stop=(kt == KT - 1),
                                tile_position=(0, po),
                            )

                    def emit_proj(b):
                        # output projection for batch b, interleaved with the
                        # next batch's attention; psum slots borrowed from the
                        # dn pool (same tag -> shared banks).
                        for nt in range(KT):
                            ps = dn_ps.tile([P, 512], F32, name="dnb", tag="dnb")
                            for cc in range(DC):
                                nc.tensor.matmul(
                                    ps,
                                    lhsT=A_sb[:, b * DC + cc, nt * P:(nt + 1) * P],
                                    rhs=wp_sb[:, cc, :],
                                    start=(cc == 0),
                                    stop=(cc == DC - 1),
                                )
                            fo = sb2.tile([P, 512], F32, name="rb", tag="rb")
                            nc.vector.tensor_tensor(fo, ps, bp_sb, ALU.add)
                            nc.sync.dma_start(out_d[b, nt * P:(nt + 1) * P, :],
                                              fo)

                    def emit_attention(hp, b, qh):
                        pv = pv_ps.tile([P, 512], F32, name="pv")
                        dnb = dn_ps.tile([P, 512], F32, name="dnb")
                        # software pipeline: QK/exp/mask one k-tile ahead of
                        # dn/PV.
                        pending = None
                        for kt in range(KT):
                            et = emit_qk(b, hp, qh, kt)
                            ex, mk = emit_exp_mask(et, qh, kt)
                            if pending is not None:
                                emit_dn_pv(b, hp, qh, *pending)
                            pending = (kt, dnb, pv, ex, mk)
                        emit_dn_pv(b, hp, qh, *pending)
                        rb = sb2.tile([P, 512], F32, name="rb")
                        nc.vector.reciprocal(rb, dnb)
                        nc.vector.tensor_tensor(
                            A_sb[:, b * DC + hp, qh * 512:(qh + 1) * 512],
                            pv, rb, ALU.mult
                        )

                    emit_v_proj()
                    for oc in range(DC):
                        emit_qk_proj(oc)
                        for b in range(NB):
                            for qh in range(2):
                                emit_attention(oc, b, qh)
                    for b in range(NB):
                        emit_proj(b)

            if reps == 1:
                emit_phases()
            else:
                with tc.For_i(0, reps, 1):
                    emit_phases()

    return nc


def make_in_maps(inputs):
    bf = ml_dtypes.bfloat16
    x = np.asarray(inputs["x"], np.float32)
    mask = np.asarray(inputs["mask"], np.float32)
    shared = {
        "WqT": np.ascontiguousarray(np.asarray(inputs["Wq"], np.float32).T).astype(bf),
        "WkT": np.ascontiguousarray(np.asarray(inputs["Wk"], np.float32).T).astype(bf),
        "WvT": np.ascontiguousarray(np.asarray(inputs["Wv"], np.float32).T).astype(bf),
        "WpT": np.ascontiguousarray(np.asarray(inputs["Wp"], np.float32).T).astype(bf),
        "maskT": np.ascontiguousarray(mask.T).astype(bf),
        "bq2": np.ascontiguousarray(np.asarray(inputs["bq"], np.float32).reshape(DC, P).T),
        "bk2": np.ascontiguousarray(np.asarray(inputs["bk"], np.float32).reshape(DC, P).T),
        "bv_rep": np.tile(np.asarray(inputs["bv"], np.float32)[None, :], (P, 1)),
        "bp_rep": np.tile(np.asarray(inputs["bp"], np.float32)[None, :], (P, 1)),
    }
    in_maps = []
    for c in range(NCORES):
        xT = np.ascontiguousarray(
            x[NB * c: NB * (c + 1)].reshape(T, D).T
        ).astype(bf)
        in_maps.append({"xT": xT, **shared})
    return in_maps


def kernel(x, mask, Wq, bq, Wk, bk, Wv, bv, Wp, bp):
    if "nc" not in _cache:
        _cache["nc"] = _build()
    nc = _cache["nc"]

    in_maps = make_in_maps(dict(x=x, mask=mask, Wq=Wq, bq=bq, Wk=Wk, bk=bk,
                                Wv=Wv, bv=bv, Wp=Wp, bp=bp))

    trace_dir = os.environ.get("BASS_TRACE_DIR")
    if trace_dir:
        import concourse.bass_utils as bu
        bu.upload_artifacts = lambda tmpdir: "local"
        res = run_bass_kernel_spmd(
            nc, in_maps, core_ids=list(range(NCORES)), trace=True,
            tmpdir=trace_dir,
        )
        kernel.last_exec_time_ns = res.exec_time_ns
        kernel.last_results = res
    else:
        res = run_bass_kernel_spmd(nc, in_maps, core_ids=list(range(NCORES)))

    outs = [np.asarray(r["out"], np.float32) for r in res.results]
    return np.concatenate(outs, axis=0)

